# revision 38
# baseline (speedup 1.0000x reference)
"""Trainium2 Bass kernel for the H2+ ion PINN loss (nn_NN_ion_52347061403910).

Math: psi = dec(R)*g(f1,f2) + f1 + f2 with f_i = exp(-r_i) and g the
symmetrized 2-16-16-1 MLP head.  The Laplacian needs (g, g1, g2) plus the
Hessian quadratic form  Q:Hg  with Q = w w^T + u u^T (w = (f1, f2*c),
u = (0, f2*s)), evaluated by tangent propagation through the tanh half-angle
form of the sigmoids (sig = (1+tanh(z/2))/2), so sig'/sig'' are polynomial in
tau = tanh.  E(R), dec(R) are runtime-fitted Chebyshev polynomials.

v2 layout: 8 cores pure data-parallel, 125000 pts/core, column-major padded
to 128 x 977, two column chunks [512 | 465].  Pointwise geometry on
[128,npc] tiles; the 16-wide MLP packs 4 point-rows x 2 branches x 16 = 128
partitions; 32 bands of 4 rows per chunk, 3-stage software-pipelined.  The
band-layout gather (F1/F2/F2c -> [12, 32*npc]) and head-output scatter
(pH rows 32j+8pb+h -> point layout) run as a handful of large DMAs via
DRAM bounces (SBUF-side APs stay plain; all index permutation happens in
DRAM->DRAM legs whose APs are unconstrained), replacing ~260 small SBUF
DMAs per chunk whose HWDGE dispatch (~650ns each) dominated v1.  All PSUM
tiles are allocated bank-aligned ([128,512] f32) and sliced to npc so
accumulation-group zero-regions never straddle generations.  Elementwise
work is spread across DVE (f16 2x/4x modes), Act, and Pool (tensor_tensor
only - no PSUM port, no TensorScalar opcode on gpsimd).  Host sends
x/y/z/R as f16 (halves tunnel transfer).  Boundary term (psi at 2x8192
indices) is computed host-side in float64.
"""

import numpy as np
from contextlib import ExitStack

import concourse.bass as bass
from concourse import bacc
import concourse.tile as tile
import concourse.mybir as mybir
from concourse.bass_utils import run_bass_kernel_spmd

F32 = mybir.dt.float32
F16 = mybir.dt.float16
AT = mybir.ActivationFunctionType
OP = mybir.AluOpType

N_CORES = 8
N_TOTAL = 1_000_000
PER_CORE = N_TOTAL // N_CORES   # 125000
NROWS = 128
NF = 977                        # columns; 128*977 = 125056 >= 125000
CHUNKS = (512, 465)
NVALID_LASTCOL = PER_CORE - (NF - 1) * NROWS  # 72 valid rows in col 976
DEG_E = 8
DEG_D = 8
CHEB_COLS = 3 + (DEG_E + 1) + (DEG_D + 1)
NBANDS = NROWS // 4             # 32
NGROUPS = NBANDS // 4           # 8 groups of 4 bands


def _sigmoid(x):
    return 1.0 / (1.0 + np.exp(-x))


def _cheb_fit(f, lo, hi, deg):
    k = np.arange(deg + 1)
    tn = np.cos((2 * k + 1) * np.pi / (2 * (deg + 1)))
    y = f(0.5 * (tn + 1) * (hi - lo) + lo)
    c = np.polynomial.chebyshev.chebfit(tn, y, deg)
    pc = np.polynomial.chebyshev.cheb2poly(c)   # power basis in t = a*R+b
    tg = np.linspace(-1, 1, 4097)
    rg = 0.5 * (tg + 1) * (hi - lo) + lo
    err = np.abs(np.polynomial.polynomial.polyval(tg, pc) - f(rg)).max()
    return pc, err


# fp16 matmul weights; fp32 biases/scalars
W16 = ("WA", "WS", "W2bd", "WD2bdN", "WCpos",
       "HG", "HVT2", "HVT", "HR1", "HR2", "HR22")
WEIGHT_SHAPES = dict(WA=(12, 128), WS=(12, 128), W2bd=(128, 128),
                     WD2bdN=(128, 128), WCpos=(128, 128),
                     HG=(128, 32), HVT2=(128, 32), HVT=(128, 32),
                     HR1=(128, 32), HR2=(128, 32), HR22=(128, 32),
                     BT1=(128, 1), BT2=(128, 1), UB0N=(128, 1))


def build_consts(params):
    """Host-side folded weight tensors (lhsT layout [K, M])."""
    p = {k: np.asarray(v, np.float64) for k, v in params.items()}
    W1 = p["W_H1"]            # [16,2]
    b1 = p["b_H1"]
    W2 = p["W_H2"]            # [16,16]
    b2 = p["b_H2"]
    Wo = p["W_out"][0]        # [16]
    w0, w1 = W1[:, 0], W1[:, 1]

    def wab(br):
        return (w0, w1) if br == 0 else (w1, w0)

    WA = np.zeros((12, 128))    # rhs rows: F1 x4, F2 x4, F2c x4
    WS = np.zeros((12, 128))    # pS = -s1/4, s1 = wa*F1 + wb*F2c
    for pb in range(4):
        for br in range(2):
            wa, wb = wab(br)
            cols = slice(32 * pb + 16 * br, 32 * pb + 16 * br + 16)
            WA[pb, cols] = wa
            WA[4 + pb, cols] = wb
            WS[pb, cols] = -wa / 4
            WS[8 + pb, cols] = -wb / 4

    W2bd = np.zeros((128, 128))   # z2 preact / tangent: out = W2 @ rhs
    WD2bdN = np.zeros((128, 128))  # pD2 = 0.25*W2 (wb * sp1), rhs = tsq1-1
    WCpos = np.zeros((128, 128))  # pC' = +0.25*W2^T (Wo * rhs)
    for pb in range(4):
        for br in range(2):
            o = 32 * pb + 16 * br
            wa, wb = wab(br)
            W2bd[o:o + 16, o:o + 16] = W2.T
            WD2bdN[o:o + 16, o:o + 16] = -0.25 * (W2 * wb[None, :]).T
            WCpos[o:o + 16, o:o + 16] = 0.25 * (Wo[:, None] * W2)

    # heads: rows of pH = 8*pb + h, h in [G, g1, g2, hw, g22]; rows 8pb+5..7
    # stay zero so the matmul initializes the full 32-row PSUM block, and
    # head h sits at uniform partition stride 8 (offset h) for the scatter.
    HG = np.zeros((128, 32))
    HVT2 = np.zeros((128, 32))
    HVT = np.zeros((128, 32))
    HR1 = np.zeros((128, 32))
    HR2 = np.zeros((128, 32))
    HR22 = np.zeros((128, 32))
    for pb in range(4):
        for br in range(2):
            r = slice(32 * pb + 16 * br, 32 * pb + 16 * br + 16)
            wa, wb = wab(br)
            HG[r, 8 * pb + 0] = 0.5 * Wo
            HVT2[r, 8 * pb + 1] = 0.25 * wa
            HVT2[r, 8 * pb + 2] = 0.25 * wb
            HVT[r, 8 * pb + 4] = -0.25 * wb * wb
            HR1[r, 8 * pb + 3] = 0.25 * Wo
            HR2[r, 8 * pb + 3] = -4.0
            HR22[r, 8 * pb + 4] = 0.25 * Wo

    BT1 = np.tile(b1 / 2, 8)[:, None]
    BT2 = np.tile((b2 + 0.5 * W2.sum(1)) / 2, 8)[:, None]
    UB0N = np.tile(np.tile(-0.25 * (Wo @ W2), 2), 4)[:, None]

    consts = dict(WA=WA, WS=WS, W2bd=W2bd, WD2bdN=WD2bdN, WCpos=WCpos,
                  HG=HG, HVT2=HVT2, HVT=HVT, HR1=HR1, HR2=HR2, HR22=HR22,
                  BT1=BT1, BT2=BT2, UB0N=UB0N)
    return {k: np.ascontiguousarray(v, np.float16 if k in W16 else np.float32)
            for k, v in consts.items()}


def build_cheb(params, R):
    """[128, CHEB_COLS]: cols [alpha, beta, c0, cE..., cD...]."""
    p = {k: np.asarray(v, np.float64) for k, v in params.items()}

    def E_of(r):
        e = _sigmoid(np.outer(r, p["W_E1"][:, 0]) + p["b_E1"])
        e = _sigmoid(e @ p["W_E2"].T + p["b_E2"])
        return e @ p["W_Eout"][0] + p["b_Eout"][0]

    def D_of(r):
        fd = _sigmoid(np.outer(r, p["W_DL"][:, 0]) + p["b_DL"])
        return fd @ p["W_D"][0] + p["b_D"][0]

    lo = float(np.min(R)) - 1e-5
    hi = float(np.max(R)) + 1e-5
    alpha = 2.0 / (hi - lo)
    beta = -(hi + lo) / (hi - lo)
    cE, eE = _cheb_fit(E_of, lo, hi, DEG_E)
    cD, eD = _cheb_fit(D_of, lo, hi, DEG_D)
    assert eE < 1e-3 and eD < 1e-3, (eE, eD)
    c0 = float(p["b_out"][0] + p["W_out"][0].sum())
    row = np.concatenate([[alpha, beta, c0], cE, cD])
    assert row.shape[0] == CHEB_COLS
    return np.ascontiguousarray(np.tile(row[None, :], (128, 1)), np.float32)


def build_bass(bench_repeat=1):
    nc = bacc.Bacc("TRN2", target_bir_lowering=False, debug=False)

    X = nc.dram_tensor("X", [NROWS, NF], F16, kind="ExternalInput")
    Y = nc.dram_tensor("Y", [NROWS, NF], F16, kind="ExternalInput")
    Z = nc.dram_tensor("Z", [NROWS, NF], F16, kind="ExternalInput")
    RT = nc.dram_tensor("RT", [NROWS, NF], F16, kind="ExternalInput")
    CHEB = nc.dram_tensor("CHEB", [NROWS, CHEB_COLS], F32, kind="ExternalInput")
    MCOL = nc.dram_tensor("MCOL", [NROWS, 1], F32, kind="ExternalInput")
    Wd = {nm: nc.dram_tensor(nm, list(shp), F16 if nm in W16 else F32,
                             kind="ExternalInput")
          for nm, shp in WEIGHT_SHAPES.items()}
    ACC_D = nc.dram_tensor("ACC", [NROWS, 1], F32, kind="ExternalOutput")

    v = nc.vector
    a = nc.scalar
    g = nc.gpsimd
    te = nc.tensor
    dma = nc.sync

    with tile.TileContext(nc) as tc, ExitStack() as ctx:
        cpool = ctx.enter_context(tc.tile_pool(name="consts", bufs=1))
        pw = ctx.enter_context(tc.tile_pool(name="pw", bufs=2))
        ft = ctx.enter_context(tc.tile_pool(name="ft", bufs=3))
        rhp = ctx.enter_context(tc.tile_pool(name="rhp", bufs=1))
        hsp = ctx.enter_context(tc.tile_pool(name="hsp", bufs=1))
        drp = ctx.enter_context(tc.tile_pool(name="drp", bufs=2, space="DRAM"))
        psA = ctx.enter_context(tc.tile_pool(name="psA", bufs=1, space="PSUM"))
        psS = ctx.enter_context(tc.tile_pool(name="psS", bufs=2, space="PSUM"))
        psC = ctx.enter_context(tc.tile_pool(name="psC", bufs=2, space="PSUM"))
        psTB = ctx.enter_context(tc.tile_pool(name="psTB", bufs=1, space="PSUM"))
        psH = ctx.enter_context(tc.tile_pool(name="psH", bufs=1, space="PSUM"))

        W = {}
        for nm in Wd:
            W[nm] = cpool.tile(list(WEIGHT_SHAPES[nm]),
                               F16 if nm in W16 else F32,
                               name=f"w_{nm}", tag=f"w_{nm}")
            dma.dma_start(W[nm][:], Wd[nm][:])
        CH = cpool.tile([NROWS, CHEB_COLS], F32, name="cheb", tag="cheb")
        dma.dma_start(CH[:], CHEB[:])
        MC = cpool.tile([NROWS, 1], F32, name="mcol", tag="mcol")
        dma.dma_start(MC[:], MCOL[:])

        def chb(i):
            return CH[:, i:i + 1]

        def body():
            acc_parts = []

            # ---------- pointwise geometry + cheb for one chunk ----------
            def pw_phase(c, npc, cs):
                def pwt(tag, dt=F32):
                    return pw.tile([NROWS, npc], dt, name=tag, tag=tag)

                env = {"c": c, "npc": npc}
                X16, Y16, Z16, R16 = (pw.tile([NROWS, npc], F16, name=t, tag=t)
                                      for t in ("X16", "Y16", "Z16", "R16"))
                dma.dma_start(X16[:], X[:, cs])
                dma.dma_start(Y16[:], Y[:, cs])
                dma.dma_start(Z16[:], Z[:, cs])
                dma.dma_start(R16[:], RT[:, cs])

                s_a, s_b, s_c = pwt("s_a"), pwt("s_b"), pwt("s_c")
                D1t = pw.tile([NROWS, npc], F16, name="D1t", tag="D1t")
                D2t = pw.tile([NROWS, npc], F16, name="D2t", tag="D2t")
                YZ2 = pwt("YZ2")
                g.tensor_sub(D1t[:], X16[:], R16[:])
                g.tensor_add(D2t[:], X16[:], R16[:])
                a.square(s_a[:], Y16[:])
                a.square(s_b[:], Z16[:])
                g.tensor_add(YZ2[:], s_a[:], s_b[:])
                R1t, R2t = pwt("R1t"), pwt("R2t")
                a.square(s_a[:], D1t[:])
                g.tensor_add(s_a[:], s_a[:], YZ2[:])
                a.sqrt(R1t[:], s_a[:])
                a.square(s_b[:], D2t[:])
                g.tensor_add(s_b[:], s_b[:], YZ2[:])
                a.sqrt(R2t[:], s_b[:])
                Q1t, Q2t = pwt("Q1t"), pwt("Q2t")
                v.reciprocal_approx_fast(out=Q1t[:], in_=R1t[:])
                v.reciprocal_approx_fast(out=Q2t[:], in_=R2t[:])
                F1t, F2t = pwt("F1t"), pwt("F2t")
                a.activation(F1t[:], R1t[:], AT.Exp, scale=-1.0)
                a.activation(F2t[:], R2t[:], AT.Exp, scale=-1.0)
                # FALL: [F1h | F2h | F2Ch] f16, feeds the band gather
                FALL = pw.tile([NROWS, 3 * npc], F16, name="FALL", tag="FALL")
                a.copy(FALL[:, 0:npc], F1t[:])
                v.tensor_copy(FALL[:, npc:2 * npc], F2t[:])
                # c12 = (D1*D2 + YZ2) * Q1 * Q2 ; F2C = F2 * c12
                g.tensor_mul(s_a[:], D1t[:], D2t[:])
                g.tensor_add(s_a[:], s_a[:], YZ2[:])
                v.tensor_mul(s_b[:], Q1t[:], Q2t[:])
                g.tensor_mul(s_c[:], s_a[:], s_b[:])
                g.tensor_mul(FALL[:, 2 * npc:3 * npc], F2t[:], s_c[:])
                env.update(F1t=F1t, F2t=F2t, FALL=FALL, R16=R16,
                           Q1t=Q1t, Q2t=Q2t)
                return env

            # assembly-only pointwise work, emitted after the gather so the
            # bands start sooner and this fills engine idle during them
            def pw_aux(env):
                npc = env["npc"]
                F1t, F2t, FALL = env["F1t"], env["F2t"], env["FALL"]
                Q1t, Q2t, R16 = env["Q1t"], env["Q2t"], env["R16"]

                def pwt(tag, dt=F32):
                    return pw.tile([NROWS, npc], dt, name=tag, tag=tag)

                s_a, s_b = pwt("as_b"), pwt("as_c")
                W2SSt = pwt("W2SSt")
                a.square(s_a[:], F2t[:])
                a.square(s_b[:], FALL[:, 2 * npc:3 * npc])
                g.tensor_sub(W2SSt[:], s_a[:], s_b[:])
                S1Lt, S2Lt = pwt("S1Lt"), pwt("S2Lt")
                v.tensor_scalar(s_a[:], Q1t[:], -2.0, 1.0, OP.mult, OP.add)
                g.tensor_mul(S1Lt[:], s_a[:], F1t[:])
                v.tensor_scalar(s_b[:], Q2t[:], -2.0, 1.0, OP.mult, OP.add)
                g.tensor_mul(S2Lt[:], s_b[:], F2t[:])
                POTEt = pwt("POTEt")
                g.tensor_add(POTEt[:], Q1t[:], Q2t[:])

                RN = pwt("RN")
                v.tensor_scalar(RN[:], R16[:], chb(0), chb(1), OP.mult, OP.add)
                EEt, DECt = pwt("as_b"), pwt("DECt")

                def horner(eng, out, base, deg):
                    eng.tensor_scalar_mul(out[:], RN[:], chb(base + deg))
                    for k in range(deg - 1, 0, -1):
                        eng.scalar_tensor_tensor(out[:], out[:], chb(base + k),
                                                 RN[:], OP.add, OP.mult)
                    eng.tensor_scalar_add(out[:], out[:], chb(base))

                horner(v, EEt, 3, DEG_E)
                horner(v, DECt, 3 + DEG_E + 1, DEG_D)
                v.tensor_add(POTEt[:], POTEt[:], EEt[:])
                env.update(W2SSt=W2SSt, S1Lt=S1Lt, S2Lt=S2Lt, POTEt=POTEt,
                           DECt=DECt)
                return env

            # ---------- band-layout gather via DRAM bounce ----------
            # RHall[4q+r, b*npc+c] = FALL[4b+r, q*npc+c].  SBUF-side APs are
            # plain (dep tracking mishandles strided SBUF reads); the index
            # permutation runs in DRAM->DRAM legs (one per q, 3-dim APs).
            def gather_a(env):
                npc = env["npc"]
                DFA = drp.tile([NROWS, 3 * npc], F16, name="DFA", tag="DFA")
                dma.dma_start(DFA[:], env["FALL"][:])
                DFB = drp.tile([12, NBANDS * npc], F16, name="DFB", tag="DFB")
                dfa_q = DFA[:].rearrange("(b r) (q c) -> q r b c",
                                         b=NBANDS, q=3)
                dfb_q = DFB[:].rearrange("(q r) (b c) -> q r b c",
                                         q=3, b=NBANDS)
                for q in range(3):
                    dma.dma_start(dfb_q[q], dfa_q[q])
                env["DFB"] = DFB

            def gather_b(env):
                npc = env["npc"]
                RHall = rhp.tile([12, NBANDS * npc], F16, name="RHall",
                                 tag="RHall")
                dma.dma_start(RHall[:], env["DFB"][:])
                env["RHall"] = RHall

            # ---------- feature bands (3-stage software pipeline) ----------
            def band_s1(env, b):
                npc = env["npc"]
                rh = env["RHall"][:, b * npc:(b + 1) * npc]

                def ftt(tag, w=1):
                    return ft.tile([128, w * npc], F16, name=tag, tag=tag)

                pA = psA.tile([128, 512], F32, name="pA", tag="pA")
                te.matmul(pA[:, 0:npc], W["WA"][:], rh, start=True, stop=True)
                pS = psS.tile([128, 512], F32, name="pS", tag="pS")
                te.matmul(pS[:, 0:npc], W["WS"][:], rh, start=True, stop=True)
                TT1 = ftt("TT1", 2)   # [T1 | TAU1]
                TAU1 = TT1[:, npc:2 * npc]
                a.activation(TAU1, pA[:, 0:npc], AT.Tanh,
                             bias=W["BT1"][:, 0:1], scale=0.5)
                TSQ1 = ftt("TSQ1")
                v.tensor_mul(TSQ1[:], TAU1, TAU1)
                SP1N = ftt("SP1N")    # tsq1 - 1 = -4 sig'(z1)
                v.tensor_scalar_sub(SP1N[:], TSQ1[:], 1.0)
                # T1 = (tsq1 - 1) * pS
                v.scalar_tensor_tensor(TT1[:, 0:npc], TSQ1[:], 1.0,
                                       pS[:, 0:npc], OP.subtract, OP.mult)
                SQS = ftt("SQS")      # (s1/4)^2
                a.square(SQS[:], pS[:, 0:npc])
                return dict(b=b, TT1=TT1, TAU1=TAU1, TSQ1=TSQ1, SP1N=SP1N,
                            SQS=SQS)

            def band_s2(env, st):
                npc = env["npc"]

                def ftt(tag, w=1):
                    return ft.tile([128, w * npc], F16, name=tag, tag=tag)

                pTB = psTB.tile([128, 1024], F32, name="pTB", tag="pTB")
                te.matmul(pTB[:, 0:npc], W["W2bd"][:], st["TT1"][:, 0:npc],
                          start=True, stop=True)
                te.matmul(pTB[:, 512:512 + npc], W["W2bd"][:],
                          st["TT1"][:, npc:2 * npc], start=True, stop=True)
                TAU2 = ftt("TAU2")
                a.activation(TAU2[:], pTB[:, 512:512 + npc], AT.Tanh,
                             bias=W["BT2"][:, 0:1], scale=0.25)
                # pD2 reuses psTB bank 1 once TAU2 has consumed pB; for the
                # 512-wide chunk one Act square covers [pT | pD2] contiguously
                te.matmul(pTB[:, 512:512 + npc], W["WD2bdN"][:],
                          st["SP1N"][:], start=True, stop=True)
                SQTD = ft.tile([128, 512 + npc], F16, name="SQTD",
                               tag="SQTD")
                if npc == 512:
                    a.square(SQTD[:], pTB[:, 0:512 + npc])
                else:
                    a.square(SQTD[:, 0:npc], pTB[:, 0:npc])
                    a.square(SQTD[:, 512:512 + npc], pTB[:, 512:512 + npc])
                SQT = SQTD[:, 0:npc]
                SQD = SQTD[:, 512:512 + npc]
                TSQ2 = ftt("TSQ2")
                v.tensor_mul(TSQ2[:], TAU2[:], TAU2[:])
                UT = ftt("UT")        # (tsq2-1)*tau2 = 4 sig''(z2)
                v.scalar_tensor_tensor(UT[:], TSQ2[:], 1.0, TAU2[:],
                                       OP.subtract, OP.mult)
                pC = psC.tile([128, 512], F32, name="pC", tag="pC")
                te.matmul(pC[:, 0:npc], W["WCpos"][:], TSQ2[:],
                          start=True, stop=True)
                st.update(SQT=SQT, TAU2=TAU2, SQD=SQD, UT=UT, pC=pC)
                return st

            def band_s3(env, st):
                npc = env["npc"]
                b = st["b"]
                gi, j = b // 4, b % 4

                def ftt(tag, w=1):
                    return ft.tile([128, w * npc], F16, name=tag, tag=tag)

                VT2 = ftt("VT2")      # (pC' + ub0n)*sp1neg = ubar*sp1
                v.scalar_tensor_tensor(VT2[:], st["pC"][:, 0:npc],
                                       W["UB0N"][:, 0:1], st["SP1N"][:],
                                       OP.add, OP.mult)
                VT = ftt("VT")
                g.tensor_mul(VT[:], VT2[:], st["TAU1"])
                R1 = ftt("R1")
                v.tensor_mul(R1[:], st["UT"][:], st["SQT"])
                R2 = ftt("R2")
                v.tensor_mul(R2[:], VT[:], st["SQS"][:])
                R22 = ftt("R22")
                g.tensor_mul(R22[:], st["UT"][:], st["SQD"])

                if j == 0:
                    env["pH"] = psH.tile([128, 512], F32, name="pH", tag="pH")
                pH = env["pH"][32 * j:32 * j + 32, 0:npc]
                tp = (0, 32 * j)
                te.matmul(pH, W["HG"][:], st["TAU2"][:], start=True,
                          stop=False, tile_position=tp)
                te.matmul(pH, W["HVT2"][:], VT2[:], start=False,
                          stop=False, tile_position=tp)
                te.matmul(pH, W["HVT"][:], VT[:], start=False, stop=False,
                          tile_position=tp)
                te.matmul(pH, W["HR1"][:], R1[:], start=False, stop=False,
                          tile_position=tp)
                te.matmul(pH, W["HR2"][:], R2[:], start=False, stop=False,
                          tile_position=tp)
                te.matmul(pH, W["HR22"][:], R22[:], start=False, stop=True,
                          tile_position=tp)
                if j == 3:
                    a.copy(env["HST"][:, gi * npc:(gi + 1) * npc],
                           env["pH"][:, 0:npc])

            def bands(env):
                npc = env["npc"]
                env["HST"] = hsp.tile([NROWS, NGROUPS * npc], F16, name="HST",
                                      tag="HST")
                win = []
                for b in range(NBANDS):
                    win.append(band_s1(env, b))
                    if len(win) >= 3:
                        band_s3(env, win.pop(0))
                    if len(win) >= 2:
                        band_s2(env, win[-2])
                band_s2(env, win[-1])
                band_s3(env, win.pop(0))
                band_s3(env, win.pop(0))

            # ---------- head-output scatter via DRAM bounce ----------
            # GALL[16g+jp, h*npc+c] = HST[8*jp+h, g*npc+c]; permutation in
            # 5 per-head DRAM->DRAM legs, SBUF sides plain.
            def scatter(env):
                npc = env["npc"]
                DSA = drp.tile([NROWS, NGROUPS * npc], F16, name="DSA",
                               tag="DSA")
                dma.dma_start(DSA[:], env["HST"][:])
                DSB = drp.tile([NROWS, 5 * npc], F16, name="DSB", tag="DSB")
                dsa_h = DSA[:].rearrange("(jp e) (g c) -> e jp g c",
                                         e=8, g=NGROUPS)
                dsb_h = DSB[:].rearrange("(g jp) (h c) -> h jp g c",
                                         g=NGROUPS, h=5)
                for h in range(5):
                    dma.dma_start(dsb_h[h], dsa_h[h])
                GALL = pw.tile([NROWS, 5 * npc], F16, name="GALL", tag="GALL")
                dma.dma_start(GALL[:], DSB[:])
                env["GALL"] = GALL

            # ---------- assembly ----------
            def assembly(env):
                c, npc = env["c"], env["npc"]
                GALL = env["GALL"]
                Gh = GALL[:, 0:npc]
                G1h = GALL[:, npc:2 * npc]
                G2h = GALL[:, 2 * npc:3 * npc]
                HWh = GALL[:, 3 * npc:4 * npc]
                G22h = GALL[:, 4 * npc:5 * npc]

                def pwt(tag, dt=F32):
                    return pw.tile([NROWS, npc], dt, name=tag, tag=tag)

                s_a, s_b, s_c = pwt("as_a"), pwt("as_b"), pwt("as_c")
                PSIt, LAPt = pwt("PSIt"), pwt("LAPt")
                v.tensor_scalar_add(s_a[:], Gh, chb(2))
                g.tensor_mul(PSIt[:], s_a[:], env["DECt"][:])
                v.tensor_add(PSIt[:], PSIt[:], env["F1t"][:])
                v.tensor_add(PSIt[:], PSIt[:], env["F2t"][:])
                v.tensor_mul(s_a[:], env["W2SSt"][:], G22h)
                g.tensor_add(s_a[:], s_a[:], HWh)
                v.tensor_mul(s_b[:], G1h, env["S1Lt"][:])
                g.tensor_add(s_a[:], s_a[:], s_b[:])
                v.tensor_mul(s_c[:], G2h, env["S2Lt"][:])
                g.tensor_add(s_a[:], s_a[:], s_c[:])
                v.tensor_mul(LAPt[:], env["DECt"][:], s_a[:])
                g.tensor_add(LAPt[:], LAPt[:], env["S1Lt"][:])
                v.tensor_add(LAPt[:], LAPt[:], env["S2Lt"][:])
                REST = pwt("REST")
                v.tensor_mul(s_a[:], env["POTEt"][:], PSIt[:])
                v.scalar_tensor_tensor(REST[:], LAPt[:], -0.5, s_a[:],
                                       OP.mult, OP.subtract)
                if c == len(CHUNKS) - 1:
                    v.tensor_mul(REST[:, npc - 1:npc],
                                 REST[:, npc - 1:npc], MC[:, 0:1])
                acc_c = cpool.tile([NROWS, 1], F32, name=f"acc{c}",
                                   tag=f"acc{c}")
                a.activation(s_a[:], REST[:], AT.Square, accum_out=acc_c[:])
                acc_parts.append(acc_c)

            env0 = pw_phase(0, CHUNKS[0], slice(0, CHUNKS[0]))
            gather_a(env0)
            gather_b(env0)
            env1 = pw_phase(1, CHUNKS[1], slice(CHUNKS[0], NF))
            gather_a(env1)
            pw_aux(env0)
            bands(env0)
            scatter(env0)
            gather_b(env1)
            pw_aux(env1)
            assembly(env0)
            bands(env1)
            scatter(env1)
            assembly(env1)

            tot = cpool.tile([NROWS, 1], F32, name="acctot", tag="acctot")
            v.tensor_add(tot[:], acc_parts[0][:], acc_parts[1][:])
            dma.dma_start(ACC_D[:], tot[:])

        if bench_repeat > 1:
            with tc.For_i(0, bench_repeat, 1):
                body()
        else:
            body()

    nc.compile()
    return nc


def make_in_maps(inputs):
    params = {k: v for k, v in inputs.items() if k not in
              ("x", "y", "z", "R", "bIndex1", "bIndex2")}
    consts = build_consts(params)
    cheb = build_cheb(params, np.asarray(inputs["R"], np.float32))
    mcol = (np.arange(NROWS) < NVALID_LASTCOL).astype(np.float32)[:, None]

    in_maps = []
    for core in range(N_CORES):
        sl = slice(core * PER_CORE, (core + 1) * PER_CORE)

        def shard(arr, fill):
            s = np.asarray(arr, np.float32)[sl, 0]
            buf = np.full((NF, NROWS), fill, np.float32)
            buf.reshape(-1)[:PER_CORE] = s
            return np.ascontiguousarray(buf.T.astype(np.float16))

        m = dict(consts)
        m["X"] = shard(inputs["x"], 0.5)
        m["Y"] = shard(inputs["y"], 0.5)
        m["Z"] = shard(inputs["z"], 0.5)
        m["RT"] = shard(inputs["R"], 1.0)
        m["CHEB"] = cheb
        m["MCOL"] = mcol
        in_maps.append(m)
    return in_maps


def host_boundary(inputs):
    """Lbc = mean(psi[b1]^2) + mean(psi[b2]^2), float64 host computation."""
    p = {k: np.asarray(v, np.float64) for k, v in inputs.items()
         if k.startswith(("W_", "b_"))}
    idx = np.concatenate([np.asarray(inputs["bIndex1"]).astype(np.int64),
                          np.asarray(inputs["bIndex2"]).astype(np.int64)])
    x = np.asarray(inputs["x"], np.float64)[idx, 0]
    y = np.asarray(inputs["y"], np.float64)[idx, 0]
    z = np.asarray(inputs["z"], np.float64)[idx, 0]
    R = np.asarray(inputs["R"], np.float64)[idx, 0]
    r1 = np.sqrt((x - R) ** 2 + y ** 2 + z ** 2)
    r2 = np.sqrt((x + R) ** 2 + y ** 2 + z ** 2)
    f1, f2 = np.exp(-r1), np.exp(-r2)
    W1, b1 = p["W_H1"], p["b_H1"]
    W2, b2 = p["W_H2"], p["b_H2"]
    B = 0.0
    for (aa, bb) in ((f1, f2), (f2, f1)):
        h = _sigmoid(np.outer(aa, W1[:, 0]) + np.outer(bb, W1[:, 1]) + b1)
        B = B + _sigmoid(h @ W2.T + b2)
    fd = _sigmoid(np.outer(R, p["W_DL"][:, 0]) + p["b_DL"])
    dec = fd @ p["W_D"][0] + p["b_D"][0]
    psi = ((B @ p["W_out"][0]) + p["b_out"][0]) * dec + f1 + f2
    n = idx.shape[0] // 2
    return float((psi[:n] ** 2).mean() + (psi[n:] ** 2).mean())


_NC_CACHE = {}


def kernel(**inputs):
    if "nc" not in _NC_CACHE:
        _NC_CACHE["nc"] = build_bass()
    nc = _NC_CACHE["nc"]

    in_maps = make_in_maps(inputs)
    results = run_bass_kernel_spmd(nc, in_maps, core_ids=list(range(N_CORES)))
    outs = results.results

    res2 = float(sum(np.asarray(outs[c]["ACC"], np.float64).sum()
                     for c in range(N_CORES)))
    loss = res2 / N_TOTAL + host_boundary(inputs)
    return np.float32(loss)


# revision 40
# speedup vs baseline: 1.0078x; 1.0078x over previous
"""Trainium2 Bass kernel for the H2+ ion PINN loss (nn_NN_ion_52347061403910).

Math: psi = dec(R)*g(f1,f2) + f1 + f2 with f_i = exp(-r_i) and g the
symmetrized 2-16-16-1 MLP head.  The Laplacian needs (g, g1, g2) plus the
Hessian quadratic form  Q:Hg  with Q = w w^T + u u^T (w = (f1, f2*c),
u = (0, f2*s)), evaluated by tangent propagation through the tanh half-angle
form of the sigmoids (sig = (1+tanh(z/2))/2), so sig'/sig'' are polynomial in
tau = tanh.  E(R), dec(R) are runtime-fitted Chebyshev polynomials.

v2 layout: 8 cores pure data-parallel, 125000 pts/core, column-major padded
to 128 x 977, two column chunks [512 | 465].  Pointwise geometry on
[128,npc] tiles; the 16-wide MLP packs 4 point-rows x 2 branches x 16 = 128
partitions; 32 bands of 4 rows per chunk, 3-stage software-pipelined.  The
band-layout gather (F1/F2/F2c -> [12, 32*npc]) and head-output scatter
(pH rows 32j+8pb+h -> point layout) run as a handful of large DMAs via
DRAM bounces (SBUF-side APs stay plain; all index permutation happens in
DRAM->DRAM legs whose APs are unconstrained), replacing ~260 small SBUF
DMAs per chunk whose HWDGE dispatch (~650ns each) dominated v1.  All PSUM
tiles are allocated bank-aligned ([128,512] f32) and sliced to npc so
accumulation-group zero-regions never straddle generations.  Elementwise
work is spread across DVE (f16 2x/4x modes), Act, and Pool (tensor_tensor
only - no PSUM port, no TensorScalar opcode on gpsimd).  Host sends
x/y/z/R as f16 (halves tunnel transfer).  Boundary term (psi at 2x8192
indices) is computed host-side in float64.
"""

import numpy as np
from contextlib import ExitStack

import concourse.bass as bass
from concourse import bacc
import concourse.tile as tile
import concourse.mybir as mybir
from concourse.bass_utils import run_bass_kernel_spmd

F32 = mybir.dt.float32
F16 = mybir.dt.float16
AT = mybir.ActivationFunctionType
OP = mybir.AluOpType

N_CORES = 8
N_TOTAL = 1_000_000
PER_CORE = N_TOTAL // N_CORES   # 125000
NROWS = 128
NF = 977                        # columns; 128*977 = 125056 >= 125000
CHUNKS = (512, 465)
NVALID_LASTCOL = PER_CORE - (NF - 1) * NROWS  # 72 valid rows in col 976
DEG_E = 8
DEG_D = 8
CHEB_COLS = 3 + (DEG_E + 1) + (DEG_D + 1)
NBANDS = NROWS // 4             # 32
NGROUPS = NBANDS // 4           # 8 groups of 4 bands


def _sigmoid(x):
    return 1.0 / (1.0 + np.exp(-x))


def _cheb_fit(f, lo, hi, deg):
    k = np.arange(deg + 1)
    tn = np.cos((2 * k + 1) * np.pi / (2 * (deg + 1)))
    y = f(0.5 * (tn + 1) * (hi - lo) + lo)
    c = np.polynomial.chebyshev.chebfit(tn, y, deg)
    pc = np.polynomial.chebyshev.cheb2poly(c)   # power basis in t = a*R+b
    tg = np.linspace(-1, 1, 4097)
    rg = 0.5 * (tg + 1) * (hi - lo) + lo
    err = np.abs(np.polynomial.polynomial.polyval(tg, pc) - f(rg)).max()
    return pc, err


# fp16 matmul weights; fp32 biases/scalars
W16 = ("WA", "WS", "W2bd", "WD2bdN", "WCpos",
       "HG", "HVT2", "HVT", "HR1", "HR2", "HR22")
WEIGHT_SHAPES = dict(WA=(12, 128), WS=(12, 128), W2bd=(128, 128),
                     WD2bdN=(128, 128), WCpos=(128, 128),
                     HG=(128, 32), HVT2=(128, 32), HVT=(128, 32),
                     HR1=(128, 32), HR2=(128, 32), HR22=(128, 32),
                     BT1=(128, 1), BT2=(128, 1), UB0N=(128, 1))


def build_consts(params):
    """Host-side folded weight tensors (lhsT layout [K, M])."""
    p = {k: np.asarray(v, np.float64) for k, v in params.items()}
    W1 = p["W_H1"]            # [16,2]
    b1 = p["b_H1"]
    W2 = p["W_H2"]            # [16,16]
    b2 = p["b_H2"]
    Wo = p["W_out"][0]        # [16]
    w0, w1 = W1[:, 0], W1[:, 1]

    def wab(br):
        return (w0, w1) if br == 0 else (w1, w0)

    WA = np.zeros((12, 128))    # rhs rows: F1 x4, F2 x4, F2c x4
    WS = np.zeros((12, 128))    # pS = -s1/4, s1 = wa*F1 + wb*F2c
    for pb in range(4):
        for br in range(2):
            wa, wb = wab(br)
            cols = slice(32 * pb + 16 * br, 32 * pb + 16 * br + 16)
            WA[pb, cols] = wa
            WA[4 + pb, cols] = wb
            WS[pb, cols] = -wa / 4
            WS[8 + pb, cols] = -wb / 4

    W2bd = np.zeros((128, 128))   # z2 preact / tangent: out = W2 @ rhs
    WD2bdN = np.zeros((128, 128))  # pD2 = 0.25*W2 (wb * sp1), rhs = tsq1-1
    WCpos = np.zeros((128, 128))  # pC' = +0.25*W2^T (Wo * rhs)
    for pb in range(4):
        for br in range(2):
            o = 32 * pb + 16 * br
            wa, wb = wab(br)
            W2bd[o:o + 16, o:o + 16] = W2.T
            WD2bdN[o:o + 16, o:o + 16] = -0.25 * (W2 * wb[None, :]).T
            WCpos[o:o + 16, o:o + 16] = 0.25 * (Wo[:, None] * W2)

    # heads: rows of pH = 8*pb + h, h in [G, g1, g2, hw, g22]; rows 8pb+5..7
    # stay zero so the matmul initializes the full 32-row PSUM block, and
    # head h sits at uniform partition stride 8 (offset h) for the scatter.
    HG = np.zeros((128, 32))
    HVT2 = np.zeros((128, 32))
    HVT = np.zeros((128, 32))
    HR1 = np.zeros((128, 32))
    HR2 = np.zeros((128, 32))
    HR22 = np.zeros((128, 32))
    for pb in range(4):
        for br in range(2):
            r = slice(32 * pb + 16 * br, 32 * pb + 16 * br + 16)
            wa, wb = wab(br)
            HG[r, 8 * pb + 0] = 0.5 * Wo
            HVT2[r, 8 * pb + 1] = 0.25 * wa
            HVT2[r, 8 * pb + 2] = 0.25 * wb
            HVT[r, 8 * pb + 4] = -0.25 * wb * wb
            HR1[r, 8 * pb + 3] = 0.25 * Wo
            HR2[r, 8 * pb + 3] = -4.0
            HR22[r, 8 * pb + 4] = 0.25 * Wo

    BT1 = np.tile(b1 / 2, 8)[:, None]
    BT2 = np.tile((b2 + 0.5 * W2.sum(1)) / 2, 8)[:, None]
    UB0N = np.tile(np.tile(-0.25 * (Wo @ W2), 2), 4)[:, None]

    consts = dict(WA=WA, WS=WS, W2bd=W2bd, WD2bdN=WD2bdN, WCpos=WCpos,
                  HG=HG, HVT2=HVT2, HVT=HVT, HR1=HR1, HR2=HR2, HR22=HR22,
                  BT1=BT1, BT2=BT2, UB0N=UB0N)
    return {k: np.ascontiguousarray(v, np.float16 if k in W16 else np.float32)
            for k, v in consts.items()}


def build_cheb(params, R):
    """[128, CHEB_COLS]: cols [alpha, beta, c0, cE..., cD...]."""
    p = {k: np.asarray(v, np.float64) for k, v in params.items()}

    def E_of(r):
        e = _sigmoid(np.outer(r, p["W_E1"][:, 0]) + p["b_E1"])
        e = _sigmoid(e @ p["W_E2"].T + p["b_E2"])
        return e @ p["W_Eout"][0] + p["b_Eout"][0]

    def D_of(r):
        fd = _sigmoid(np.outer(r, p["W_DL"][:, 0]) + p["b_DL"])
        return fd @ p["W_D"][0] + p["b_D"][0]

    lo = float(np.min(R)) - 1e-5
    hi = float(np.max(R)) + 1e-5
    alpha = 2.0 / (hi - lo)
    beta = -(hi + lo) / (hi - lo)
    cE, eE = _cheb_fit(E_of, lo, hi, DEG_E)
    cD, eD = _cheb_fit(D_of, lo, hi, DEG_D)
    assert eE < 1e-3 and eD < 1e-3, (eE, eD)
    c0 = float(p["b_out"][0] + p["W_out"][0].sum())
    row = np.concatenate([[alpha, beta, c0], cE, cD])
    assert row.shape[0] == CHEB_COLS
    return np.ascontiguousarray(np.tile(row[None, :], (128, 1)), np.float32)


def build_bass(bench_repeat=1):
    nc = bacc.Bacc("TRN2", target_bir_lowering=False, debug=False)

    X = nc.dram_tensor("X", [NROWS, NF], F16, kind="ExternalInput")
    Y = nc.dram_tensor("Y", [NROWS, NF], F16, kind="ExternalInput")
    Z = nc.dram_tensor("Z", [NROWS, NF], F16, kind="ExternalInput")
    RT = nc.dram_tensor("RT", [NROWS, NF], F16, kind="ExternalInput")
    CHEB = nc.dram_tensor("CHEB", [NROWS, CHEB_COLS], F32, kind="ExternalInput")
    MCOL = nc.dram_tensor("MCOL", [NROWS, 1], F32, kind="ExternalInput")
    Wd = {nm: nc.dram_tensor(nm, list(shp), F16 if nm in W16 else F32,
                             kind="ExternalInput")
          for nm, shp in WEIGHT_SHAPES.items()}
    ACC_D = nc.dram_tensor("ACC", [NROWS, 1], F32, kind="ExternalOutput")

    v = nc.vector
    a = nc.scalar
    g = nc.gpsimd
    te = nc.tensor
    dma = nc.sync

    with tile.TileContext(nc) as tc, ExitStack() as ctx:
        cpool = ctx.enter_context(tc.tile_pool(name="consts", bufs=1))
        pw = ctx.enter_context(tc.tile_pool(name="pw", bufs=2))
        ft = ctx.enter_context(tc.tile_pool(name="ft", bufs=3))
        rhp = ctx.enter_context(tc.tile_pool(name="rhp", bufs=1))
        hsp = ctx.enter_context(tc.tile_pool(name="hsp", bufs=1))
        drp = ctx.enter_context(tc.tile_pool(name="drp", bufs=2, space="DRAM"))
        psA = ctx.enter_context(tc.tile_pool(name="psA", bufs=1, space="PSUM"))
        psS = ctx.enter_context(tc.tile_pool(name="psS", bufs=2, space="PSUM"))
        psC = ctx.enter_context(tc.tile_pool(name="psC", bufs=2, space="PSUM"))
        psTB = ctx.enter_context(tc.tile_pool(name="psTB", bufs=1, space="PSUM"))
        psH = ctx.enter_context(tc.tile_pool(name="psH", bufs=1, space="PSUM"))

        W = {}
        for nm in Wd:
            W[nm] = cpool.tile(list(WEIGHT_SHAPES[nm]),
                               F16 if nm in W16 else F32,
                               name=f"w_{nm}", tag=f"w_{nm}")
            dma.dma_start(W[nm][:], Wd[nm][:])
        CH = cpool.tile([NROWS, CHEB_COLS], F32, name="cheb", tag="cheb")
        dma.dma_start(CH[:], CHEB[:])
        MC = cpool.tile([NROWS, 1], F32, name="mcol", tag="mcol")
        dma.dma_start(MC[:], MCOL[:])

        def chb(i):
            return CH[:, i:i + 1]

        def body():
            acc_parts = []

            # ---------- pointwise geometry + cheb for one chunk ----------
            def pw_phase(c, npc, cs):
                def pwt(tag, dt=F32):
                    return pw.tile([NROWS, npc], dt, name=tag, tag=tag)

                env = {"c": c, "npc": npc}
                X16, Y16, Z16, R16 = (pw.tile([NROWS, npc], F16, name=t, tag=t)
                                      for t in ("X16", "Y16", "Z16", "R16"))
                dma.dma_start(X16[:], X[:, cs])
                dma.dma_start(Y16[:], Y[:, cs])
                dma.dma_start(Z16[:], Z[:, cs])
                dma.dma_start(R16[:], RT[:, cs])

                s_a, s_b, s_c = pwt("s_a"), pwt("s_b"), pwt("s_c")
                D1t = pw.tile([NROWS, npc], F16, name="D1t", tag="D1t")
                D2t = pw.tile([NROWS, npc], F16, name="D2t", tag="D2t")
                YZ2 = pwt("YZ2")
                g.tensor_sub(D1t[:], X16[:], R16[:])
                g.tensor_add(D2t[:], X16[:], R16[:])
                a.square(s_a[:], Y16[:])
                a.square(s_b[:], Z16[:])
                g.tensor_add(YZ2[:], s_a[:], s_b[:])
                R1t, R2t = pwt("R1t"), pwt("R2t")
                a.square(s_a[:], D1t[:])
                g.tensor_add(s_a[:], s_a[:], YZ2[:])
                a.sqrt(R1t[:], s_a[:])
                a.square(s_b[:], D2t[:])
                g.tensor_add(s_b[:], s_b[:], YZ2[:])
                a.sqrt(R2t[:], s_b[:])
                Q1t, Q2t = pwt("Q1t"), pwt("Q2t")
                v.reciprocal_approx_fast(out=Q1t[:], in_=R1t[:])
                v.reciprocal_approx_fast(out=Q2t[:], in_=R2t[:])
                F1t, F2t = pwt("F1t"), pwt("F2t")
                a.activation(F1t[:], R1t[:], AT.Exp, scale=-1.0)
                a.activation(F2t[:], R2t[:], AT.Exp, scale=-1.0)
                # FALL: [F1h | F2h | F2Ch] f16, feeds the band gather
                FALL = pw.tile([NROWS, 3 * npc], F16, name="FALL", tag="FALL")
                a.copy(FALL[:, 0:npc], F1t[:])
                v.tensor_copy(FALL[:, npc:2 * npc], F2t[:])
                # c12 = (D1*D2 + YZ2) * Q1 * Q2 ; F2C = F2 * c12
                g.tensor_mul(s_a[:], D1t[:], D2t[:])
                g.tensor_add(s_a[:], s_a[:], YZ2[:])
                v.tensor_mul(s_b[:], Q1t[:], Q2t[:])
                g.tensor_mul(s_c[:], s_a[:], s_b[:])
                g.tensor_mul(FALL[:, 2 * npc:3 * npc], F2t[:], s_c[:])
                env.update(F1t=F1t, F2t=F2t, FALL=FALL, R16=R16,
                           Q1t=Q1t, Q2t=Q2t)
                return env

            # assembly-only pointwise work, emitted after the gather so the
            # bands start sooner and this fills engine idle during them
            def pw_aux(env):
                npc = env["npc"]
                F1t, F2t, FALL = env["F1t"], env["F2t"], env["FALL"]
                Q1t, Q2t, R16 = env["Q1t"], env["Q2t"], env["R16"]

                def pwt(tag, dt=F32):
                    return pw.tile([NROWS, npc], dt, name=tag, tag=tag)

                s_a, s_b = pwt("as_b"), pwt("as_c")
                W2SSt = pwt("W2SSt")
                a.square(s_a[:], F2t[:])
                a.square(s_b[:], FALL[:, 2 * npc:3 * npc])
                g.tensor_sub(W2SSt[:], s_a[:], s_b[:])
                S1Lt, S2Lt = pwt("S1Lt"), pwt("S2Lt")
                v.tensor_scalar(s_a[:], Q1t[:], -2.0, 1.0, OP.mult, OP.add)
                g.tensor_mul(S1Lt[:], s_a[:], F1t[:])
                v.tensor_scalar(s_b[:], Q2t[:], -2.0, 1.0, OP.mult, OP.add)
                g.tensor_mul(S2Lt[:], s_b[:], F2t[:])
                POTEt = pwt("POTEt")
                g.tensor_add(POTEt[:], Q1t[:], Q2t[:])

                RN = pwt("RN")
                v.tensor_scalar(RN[:], R16[:], chb(0), chb(1), OP.mult, OP.add)
                EEt, DECt = pwt("as_b"), pwt("DECt")

                def horner(eng, out, base, deg):
                    eng.tensor_scalar_mul(out[:], RN[:], chb(base + deg))
                    for k in range(deg - 1, 0, -1):
                        eng.scalar_tensor_tensor(out[:], out[:], chb(base + k),
                                                 RN[:], OP.add, OP.mult)
                    eng.tensor_scalar_add(out[:], out[:], chb(base))

                horner(v, EEt, 3, DEG_E)
                horner(v, DECt, 3 + DEG_E + 1, DEG_D)
                v.tensor_add(POTEt[:], POTEt[:], EEt[:])
                env.update(W2SSt=W2SSt, S1Lt=S1Lt, S2Lt=S2Lt, POTEt=POTEt,
                           DECt=DECt)
                return env

            # ---------- band-layout gather via DRAM bounce ----------
            # RHall[4q+r, b*npc+c] = FALL[4b+r, q*npc+c].  SBUF-side APs are
            # plain (dep tracking mishandles strided SBUF reads); the index
            # permutation runs in DRAM->DRAM legs (one per q, 3-dim APs).
            def gather_a(env):
                npc = env["npc"]
                DFA = drp.tile([NROWS, 3 * npc], F16, name="DFA", tag="DFA")
                dma.dma_start(DFA[:], env["FALL"][:])
                DFB = drp.tile([12, NBANDS * npc], F16, name="DFB", tag="DFB")
                dfa_q = DFA[:].rearrange("(b r) (q c) -> q r b c",
                                         b=NBANDS, q=3)
                dfb_q = DFB[:].rearrange("(q r) (b c) -> q r b c",
                                         q=3, b=NBANDS)
                for q in range(3):
                    dma.dma_start(dfb_q[q], dfa_q[q])
                env["DFB"] = DFB

            def gather_b(env):
                npc = env["npc"]
                RHall = rhp.tile([12, NBANDS * npc], F16, name="RHall",
                                 tag="RHall")
                dma.dma_start(RHall[:], env["DFB"][:])
                env["RHall"] = RHall

            # ---------- feature bands (3-stage software pipeline) ----------
            def band_s1(env, b):
                npc = env["npc"]
                rh = env["RHall"][:, b * npc:(b + 1) * npc]

                def ftt(tag, w=1):
                    return ft.tile([128, w * npc], F16, name=tag, tag=tag)

                pA = psA.tile([128, 512], F32, name="pA", tag="pA")
                te.matmul(pA[:, 0:npc], W["WA"][:], rh, start=True, stop=True)
                pS = psS.tile([128, 512], F32, name="pS", tag="pS")
                te.matmul(pS[:, 0:npc], W["WS"][:], rh, start=True, stop=True)
                TT1 = ftt("TT1", 2)   # [T1 | TAU1]
                TAU1 = TT1[:, npc:2 * npc]
                a.activation(TAU1, pA[:, 0:npc], AT.Tanh,
                             bias=W["BT1"][:, 0:1], scale=0.5)
                TSQ1 = ftt("TSQ1")
                v.tensor_mul(TSQ1[:], TAU1, TAU1)
                SP1N = ftt("SP1N")    # tsq1 - 1 = -4 sig'(z1)
                v.tensor_scalar_sub(SP1N[:], TSQ1[:], 1.0)
                # T1 = (tsq1 - 1) * pS
                v.scalar_tensor_tensor(TT1[:, 0:npc], TSQ1[:], 1.0,
                                       pS[:, 0:npc], OP.subtract, OP.mult)
                SQS = ftt("SQS")      # (s1/4)^2
                a.square(SQS[:], pS[:, 0:npc])
                return dict(b=b, TT1=TT1, TAU1=TAU1, TSQ1=TSQ1, SP1N=SP1N,
                            SQS=SQS)

            def band_s2(env, st):
                npc = env["npc"]

                def ftt(tag, w=1):
                    return ft.tile([128, w * npc], F16, name=tag, tag=tag)

                pTB = psTB.tile([128, 1024], F32, name="pTB", tag="pTB")
                te.matmul(pTB[:, 0:npc], W["W2bd"][:], st["TT1"][:, 0:npc],
                          start=True, stop=True)
                te.matmul(pTB[:, 512:512 + npc], W["W2bd"][:],
                          st["TT1"][:, npc:2 * npc], start=True, stop=True)
                TAU2 = ftt("TAU2")
                a.activation(TAU2[:], pTB[:, 512:512 + npc], AT.Tanh,
                             bias=W["BT2"][:, 0:1], scale=0.25)
                # pD2 reuses psTB bank 1 once TAU2 has consumed pB; for the
                # 512-wide chunk one Act square covers [pT | pD2] contiguously
                te.matmul(pTB[:, 512:512 + npc], W["WD2bdN"][:],
                          st["SP1N"][:], start=True, stop=True)
                SQTD = ft.tile([128, 512 + npc], F16, name="SQTD",
                               tag="SQTD")
                if npc == 512:
                    a.square(SQTD[:], pTB[:, 0:512 + npc])
                else:
                    a.square(SQTD[:, 0:npc], pTB[:, 0:npc])
                    a.square(SQTD[:, 512:512 + npc], pTB[:, 512:512 + npc])
                SQT = SQTD[:, 0:npc]
                SQD = SQTD[:, 512:512 + npc]
                TSQ2 = ftt("TSQ2")
                v.tensor_mul(TSQ2[:], TAU2[:], TAU2[:])
                UT = ftt("UT")        # (tsq2-1)*tau2 = 4 sig''(z2)
                v.scalar_tensor_tensor(UT[:], TSQ2[:], 1.0, TAU2[:],
                                       OP.subtract, OP.mult)
                pC = psC.tile([128, 512], F32, name="pC", tag="pC")
                te.matmul(pC[:, 0:npc], W["WCpos"][:], TSQ2[:],
                          start=True, stop=True)
                st.update(SQT=SQT, TAU2=TAU2, SQD=SQD, UT=UT, pC=pC)
                return st

            def band_s3(env, st):
                npc = env["npc"]
                b = st["b"]
                gi, j = b // 4, b % 4

                def ftt(tag, w=1):
                    return ft.tile([128, w * npc], F16, name=tag, tag=tag)

                VT2 = ftt("VT2")      # (pC' + ub0n)*sp1neg = ubar*sp1
                v.scalar_tensor_tensor(VT2[:], st["pC"][:, 0:npc],
                                       W["UB0N"][:, 0:1], st["SP1N"][:],
                                       OP.add, OP.mult)
                VT = ftt("VT")
                g.tensor_mul(VT[:], VT2[:], st["TAU1"])
                R1 = ftt("R1")
                v.tensor_mul(R1[:], st["UT"][:], st["SQT"])
                R2 = ftt("R2")
                v.tensor_mul(R2[:], VT[:], st["SQS"][:])
                R22 = ftt("R22")
                g.tensor_mul(R22[:], st["UT"][:], st["SQD"])

                if j == 0:
                    env["pH"] = psH.tile([128, 512], F32, name="pH", tag="pH")
                pH = env["pH"][32 * j:32 * j + 32, 0:npc]
                tp = (0, 32 * j)
                te.matmul(pH, W["HG"][:], st["TAU2"][:], start=True,
                          stop=False, tile_position=tp)
                te.matmul(pH, W["HVT2"][:], VT2[:], start=False,
                          stop=False, tile_position=tp)
                te.matmul(pH, W["HVT"][:], VT[:], start=False, stop=False,
                          tile_position=tp)
                te.matmul(pH, W["HR1"][:], R1[:], start=False, stop=False,
                          tile_position=tp)
                te.matmul(pH, W["HR2"][:], R2[:], start=False, stop=False,
                          tile_position=tp)
                te.matmul(pH, W["HR22"][:], R22[:], start=False, stop=True,
                          tile_position=tp)
                if j == 3:
                    a.copy(env["HST"][:, gi * npc:(gi + 1) * npc],
                           env["pH"][:, 0:npc])

            def bands(env):
                npc = env["npc"]
                env["HST"] = hsp.tile([NROWS, NGROUPS * npc], F16, name="HST",
                                      tag="HST")
                win = []
                for b in range(NBANDS):
                    win.append(band_s1(env, b))
                    if len(win) >= 3:
                        band_s3(env, win.pop(0))
                    if len(win) >= 2:
                        band_s2(env, win[-2])
                band_s2(env, win[-1])
                band_s3(env, win.pop(0))
                band_s3(env, win.pop(0))

            # ---------- head-output scatter via DRAM bounce ----------
            # GALL[16g+jp, h*npc+c] = HST[8*jp+h, g*npc+c]; permutation in
            # 5 per-head DRAM->DRAM legs, SBUF sides plain.
            def scatter(env):
                npc = env["npc"]
                DSA = drp.tile([NROWS, NGROUPS * npc], F16, name="DSA",
                               tag="DSA")
                dma.dma_start(DSA[:], env["HST"][:])
                DSB = drp.tile([NROWS, 5 * npc], F16, name="DSB", tag="DSB")
                dsa_h = DSA[:].rearrange("(jp e) (g c) -> e jp g c",
                                         e=8, g=NGROUPS)
                dsb_h = DSB[:].rearrange("(g jp) (h c) -> h jp g c",
                                         g=NGROUPS, h=5)
                for h in range(5):
                    dma.dma_start(dsb_h[h], dsa_h[h])
                GALL = pw.tile([NROWS, 5 * npc], F16, name="GALL", tag="GALL")
                dma.dma_start(GALL[:], DSB[:])
                env["GALL"] = GALL

            # ---------- assembly ----------
            def assembly(env):
                c, npc = env["c"], env["npc"]
                GALL = env["GALL"]
                Gh = GALL[:, 0:npc]
                G1h = GALL[:, npc:2 * npc]
                G2h = GALL[:, 2 * npc:3 * npc]
                HWh = GALL[:, 3 * npc:4 * npc]
                G22h = GALL[:, 4 * npc:5 * npc]

                def pwt(tag, dt=F32):
                    return pw.tile([NROWS, npc], dt, name=tag, tag=tag)

                s_a, s_b, s_c = pwt("as_a"), pwt("as_b"), pwt("as_c")
                PSIt, LAPt = pwt("PSIt"), pwt("LAPt")
                v.tensor_scalar_add(s_a[:], Gh, chb(2))
                g.tensor_mul(PSIt[:], s_a[:], env["DECt"][:])
                v.tensor_add(PSIt[:], PSIt[:], env["F1t"][:])
                v.tensor_add(PSIt[:], PSIt[:], env["F2t"][:])
                v.tensor_mul(s_a[:], env["W2SSt"][:], G22h)
                g.tensor_add(s_a[:], s_a[:], HWh)
                v.tensor_mul(s_b[:], G1h, env["S1Lt"][:])
                g.tensor_add(s_a[:], s_a[:], s_b[:])
                v.tensor_mul(s_c[:], G2h, env["S2Lt"][:])
                g.tensor_add(s_a[:], s_a[:], s_c[:])
                v.tensor_mul(LAPt[:], env["DECt"][:], s_a[:])
                g.tensor_add(LAPt[:], LAPt[:], env["S1Lt"][:])
                v.tensor_add(LAPt[:], LAPt[:], env["S2Lt"][:])
                REST = pwt("REST")
                v.tensor_mul(s_a[:], env["POTEt"][:], PSIt[:])
                v.scalar_tensor_tensor(REST[:], LAPt[:], -0.5, s_a[:],
                                       OP.mult, OP.subtract)
                if c == len(CHUNKS) - 1:
                    v.tensor_mul(REST[:, npc - 1:npc],
                                 REST[:, npc - 1:npc], MC[:, 0:1])
                acc_c = cpool.tile([NROWS, 1], F32, name=f"acc{c}",
                                   tag=f"acc{c}")
                a.activation(s_a[:], REST[:], AT.Square, accum_out=acc_c[:])
                acc_parts.append(acc_c)

            env0 = pw_phase(0, CHUNKS[0], slice(0, CHUNKS[0]))
            gather_a(env0)
            gather_b(env0)
            env1 = pw_phase(1, CHUNKS[1], slice(CHUNKS[0], NF))
            gather_a(env1)
            pw_aux(env0)
            bands(env0)
            scatter(env0)
            gather_b(env1)
            pw_aux(env1)
            assembly(env0)
            bands(env1)
            scatter(env1)
            assembly(env1)

            tot = cpool.tile([NROWS, 1], F32, name="acctot", tag="acctot")
            v.tensor_add(tot[:], acc_parts[0][:], acc_parts[1][:])
            dma.dma_start(ACC_D[:], tot[:])

        if bench_repeat > 1:
            with tc.For_i(0, bench_repeat, 1):
                body()
        else:
            body()

    nc.compile()
    return nc


def make_in_maps(inputs):
    params = {k: v for k, v in inputs.items() if k not in
              ("x", "y", "z", "R", "bIndex1", "bIndex2")}
    consts = build_consts(params)
    cheb = build_cheb(params, np.asarray(inputs["R"], np.float32))
    mcol = (np.arange(NROWS) < NVALID_LASTCOL).astype(np.float32)[:, None]

    in_maps = []
    for core in range(N_CORES):
        sl = slice(core * PER_CORE, (core + 1) * PER_CORE)

        def shard(arr, fill):
            s = np.asarray(arr, np.float32)[sl, 0]
            buf = np.full((NF, NROWS), fill, np.float32)
            buf.reshape(-1)[:PER_CORE] = s
            return np.ascontiguousarray(buf.T.astype(np.float16))

        m = dict(consts)
        m["X"] = shard(inputs["x"], 0.5)
        m["Y"] = shard(inputs["y"], 0.5)
        m["Z"] = shard(inputs["z"], 0.5)
        m["RT"] = shard(inputs["R"], 1.0)
        m["CHEB"] = cheb
        m["MCOL"] = mcol
        in_maps.append(m)
    return in_maps


def host_boundary(inputs):
    """Lbc = mean(psi[b1]^2) + mean(psi[b2]^2), float64 host computation."""
    p = {k: np.asarray(v, np.float64) for k, v in inputs.items()
         if k.startswith(("W_", "b_"))}
    idx = np.concatenate([np.asarray(inputs["bIndex1"]).astype(np.int64),
                          np.asarray(inputs["bIndex2"]).astype(np.int64)])
    x = np.asarray(inputs["x"], np.float64)[idx, 0]
    y = np.asarray(inputs["y"], np.float64)[idx, 0]
    z = np.asarray(inputs["z"], np.float64)[idx, 0]
    R = np.asarray(inputs["R"], np.float64)[idx, 0]
    r1 = np.sqrt((x - R) ** 2 + y ** 2 + z ** 2)
    r2 = np.sqrt((x + R) ** 2 + y ** 2 + z ** 2)
    f1, f2 = np.exp(-r1), np.exp(-r2)
    W1, b1 = p["W_H1"], p["b_H1"]
    W2, b2 = p["W_H2"], p["b_H2"]
    B = 0.0
    for (aa, bb) in ((f1, f2), (f2, f1)):
        h = _sigmoid(np.outer(aa, W1[:, 0]) + np.outer(bb, W1[:, 1]) + b1)
        B = B + _sigmoid(h @ W2.T + b2)
    fd = _sigmoid(np.outer(R, p["W_DL"][:, 0]) + p["b_DL"])
    dec = fd @ p["W_D"][0] + p["b_D"][0]
    psi = ((B @ p["W_out"][0]) + p["b_out"][0]) * dec + f1 + f2
    n = idx.shape[0] // 2
    return float((psi[:n] ** 2).mean() + (psi[n:] ** 2).mean())


_NC_CACHE = {}


def kernel(**inputs):
    if "nc" not in _NC_CACHE:
        _NC_CACHE["nc"] = build_bass()
    nc = _NC_CACHE["nc"]

    in_maps = make_in_maps(inputs)
    results = run_bass_kernel_spmd(nc, in_maps, core_ids=list(range(N_CORES)))
    outs = results.results

    res2 = float(sum(np.asarray(outs[c]["ACC"], np.float64).sum()
                     for c in range(N_CORES)))
    loss = res2 / N_TOTAL + host_boundary(inputs)
    return np.float32(loss)


# revision 42
# speedup vs baseline: 1.0341x; 1.0261x over previous
"""Trainium2 Bass kernel for the H2+ ion PINN loss (nn_NN_ion_52347061403910).

Math: psi = dec(R)*g(f1,f2) + f1 + f2 with f_i = exp(-r_i) and g the
symmetrized 2-16-16-1 MLP head.  The Laplacian needs (g, g1, g2) plus the
Hessian quadratic form  Q:Hg  with Q = w w^T + u u^T (w = (f1, f2*c),
u = (0, f2*s)), evaluated by tangent propagation through the tanh half-angle
form of the sigmoids (sig = (1+tanh(z/2))/2), so sig'/sig'' are polynomial in
tau = tanh.  E(R), dec(R) are runtime-fitted Chebyshev polynomials.

v2 layout: 8 cores pure data-parallel, 125000 pts/core, column-major padded
to 128 x 977, two column chunks [512 | 465].  Pointwise geometry on
[128,npc] tiles; the 16-wide MLP packs 4 point-rows x 2 branches x 16 = 128
partitions; 32 bands of 4 rows per chunk, 3-stage software-pipelined.  The
band-layout gather (F1/F2/F2c -> [12, 32*npc]) and head-output scatter
(pH rows 32j+8pb+h -> point layout) run as a handful of large DMAs via
DRAM bounces (SBUF-side APs stay plain; all index permutation happens in
DRAM->DRAM legs whose APs are unconstrained), replacing ~260 small SBUF
DMAs per chunk whose HWDGE dispatch (~650ns each) dominated v1.  All PSUM
tiles are allocated bank-aligned ([128,512] f32) and sliced to npc so
accumulation-group zero-regions never straddle generations.  Elementwise
work is spread across DVE (f16 2x/4x modes), Act, and Pool (tensor_tensor
only - no PSUM port, no TensorScalar opcode on gpsimd).  Host sends
x/y/z/R as f16 (halves tunnel transfer).  Boundary term (psi at 2x8192
indices) is computed host-side in float64.
"""

import numpy as np
from contextlib import ExitStack

import concourse.bass as bass
from concourse import bacc
import concourse.tile as tile
import concourse.mybir as mybir
from concourse.bass_utils import run_bass_kernel_spmd

F32 = mybir.dt.float32
F16 = mybir.dt.float16
AT = mybir.ActivationFunctionType
OP = mybir.AluOpType

N_CORES = 8
N_TOTAL = 1_000_000
PER_CORE = N_TOTAL // N_CORES   # 125000
NROWS = 128
NF = 977                        # columns; 128*977 = 125056 >= 125000
CHUNKS = (512, 465)
NVALID_LASTCOL = PER_CORE - (NF - 1) * NROWS  # 72 valid rows in col 976
DEG_E = 8
DEG_D = 8
CHEB_COLS = 3 + (DEG_E + 1) + (DEG_D + 1)
NBANDS = NROWS // 4             # 32
NGROUPS = NBANDS // 4           # 8 groups of 4 bands


def _sigmoid(x):
    return 1.0 / (1.0 + np.exp(-x))


def _cheb_fit(f, lo, hi, deg):
    k = np.arange(deg + 1)
    tn = np.cos((2 * k + 1) * np.pi / (2 * (deg + 1)))
    y = f(0.5 * (tn + 1) * (hi - lo) + lo)
    c = np.polynomial.chebyshev.chebfit(tn, y, deg)
    pc = np.polynomial.chebyshev.cheb2poly(c)   # power basis in t = a*R+b
    tg = np.linspace(-1, 1, 4097)
    rg = 0.5 * (tg + 1) * (hi - lo) + lo
    err = np.abs(np.polynomial.polynomial.polyval(tg, pc) - f(rg)).max()
    return pc, err


# fp16 matmul weights; fp32 biases/scalars
W16 = ("WA", "WS", "W2bd", "WD2bdN", "WCpos",
       "HG", "HVT2", "HVT", "HR1", "HR2", "HR22")
WEIGHT_SHAPES = dict(WA=(12, 128), WS=(12, 128), W2bd=(128, 128),
                     WD2bdN=(128, 128), WCpos=(128, 128),
                     HG=(128, 32), HVT2=(128, 32), HVT=(128, 32),
                     HR1=(128, 32), HR2=(128, 32), HR22=(128, 32),
                     BT1=(128, 1), BT2=(128, 1), UB0N=(128, 1))


def build_consts(params):
    """Host-side folded weight tensors (lhsT layout [K, M])."""
    p = {k: np.asarray(v, np.float64) for k, v in params.items()}
    W1 = p["W_H1"]            # [16,2]
    b1 = p["b_H1"]
    W2 = p["W_H2"]            # [16,16]
    b2 = p["b_H2"]
    Wo = p["W_out"][0]        # [16]
    w0, w1 = W1[:, 0], W1[:, 1]

    def wab(br):
        return (w0, w1) if br == 0 else (w1, w0)

    WA = np.zeros((12, 128))    # rhs rows: F1 x4, F2 x4, F2c x4
    WS = np.zeros((12, 128))    # pS = -s1/4, s1 = wa*F1 + wb*F2c
    for pb in range(4):
        for br in range(2):
            wa, wb = wab(br)
            cols = slice(32 * pb + 16 * br, 32 * pb + 16 * br + 16)
            WA[pb, cols] = wa
            WA[4 + pb, cols] = wb
            WS[pb, cols] = -wa / 4
            WS[8 + pb, cols] = -wb / 4

    W2bd = np.zeros((128, 128))   # z2 preact / tangent: out = W2 @ rhs
    WD2bdN = np.zeros((128, 128))  # pD2 = 0.25*W2 (wb * sp1), rhs = tsq1-1
    WCpos = np.zeros((128, 128))  # pC' = +0.25*W2^T (Wo * rhs)
    for pb in range(4):
        for br in range(2):
            o = 32 * pb + 16 * br
            wa, wb = wab(br)
            W2bd[o:o + 16, o:o + 16] = W2.T
            WD2bdN[o:o + 16, o:o + 16] = -0.25 * (W2 * wb[None, :]).T
            WCpos[o:o + 16, o:o + 16] = 0.25 * (Wo[:, None] * W2)

    # heads: rows of pH = 8*pb + h, h in [G, g1, g2, hw, g22]; rows 8pb+5..7
    # stay zero so the matmul initializes the full 32-row PSUM block, and
    # head h sits at uniform partition stride 8 (offset h) for the scatter.
    HG = np.zeros((128, 32))
    HVT2 = np.zeros((128, 32))
    HVT = np.zeros((128, 32))
    HR1 = np.zeros((128, 32))
    HR2 = np.zeros((128, 32))
    HR22 = np.zeros((128, 32))
    for pb in range(4):
        for br in range(2):
            r = slice(32 * pb + 16 * br, 32 * pb + 16 * br + 16)
            wa, wb = wab(br)
            HG[r, 8 * pb + 0] = 0.5 * Wo
            HVT2[r, 8 * pb + 1] = 0.25 * wa
            HVT2[r, 8 * pb + 2] = 0.25 * wb
            HVT[r, 8 * pb + 4] = -0.25 * wb * wb
            HR1[r, 8 * pb + 3] = 0.25 * Wo
            HR2[r, 8 * pb + 3] = -4.0
            HR22[r, 8 * pb + 4] = 0.25 * Wo

    BT1 = np.tile(b1 / 2, 8)[:, None]
    BT2 = np.tile((b2 + 0.5 * W2.sum(1)) / 2, 8)[:, None]
    UB0N = np.tile(np.tile(-0.25 * (Wo @ W2), 2), 4)[:, None]

    consts = dict(WA=WA, WS=WS, W2bd=W2bd, WD2bdN=WD2bdN, WCpos=WCpos,
                  HG=HG, HVT2=HVT2, HVT=HVT, HR1=HR1, HR2=HR2, HR22=HR22,
                  BT1=BT1, BT2=BT2, UB0N=UB0N)
    return {k: np.ascontiguousarray(v, np.float16 if k in W16 else np.float32)
            for k, v in consts.items()}


def build_cheb(params, R):
    """[128, CHEB_COLS]: cols [alpha, beta, c0, cE..., cD...]."""
    p = {k: np.asarray(v, np.float64) for k, v in params.items()}

    def E_of(r):
        e = _sigmoid(np.outer(r, p["W_E1"][:, 0]) + p["b_E1"])
        e = _sigmoid(e @ p["W_E2"].T + p["b_E2"])
        return e @ p["W_Eout"][0] + p["b_Eout"][0]

    def D_of(r):
        fd = _sigmoid(np.outer(r, p["W_DL"][:, 0]) + p["b_DL"])
        return fd @ p["W_D"][0] + p["b_D"][0]

    lo = float(np.min(R)) - 1e-5
    hi = float(np.max(R)) + 1e-5
    alpha = 2.0 / (hi - lo)
    beta = -(hi + lo) / (hi - lo)
    cE, eE = _cheb_fit(E_of, lo, hi, DEG_E)
    cD, eD = _cheb_fit(D_of, lo, hi, DEG_D)
    assert eE < 1e-3 and eD < 1e-3, (eE, eD)
    c0 = float(p["b_out"][0] + p["W_out"][0].sum())
    row = np.concatenate([[alpha, beta, c0], cE, cD])
    assert row.shape[0] == CHEB_COLS
    return np.ascontiguousarray(np.tile(row[None, :], (128, 1)), np.float32)


def build_bass(bench_repeat=1):
    nc = bacc.Bacc("TRN2", target_bir_lowering=False, debug=False)

    X = nc.dram_tensor("X", [NROWS, NF], F16, kind="ExternalInput")
    Y = nc.dram_tensor("Y", [NROWS, NF], F16, kind="ExternalInput")
    Z = nc.dram_tensor("Z", [NROWS, NF], F16, kind="ExternalInput")
    RT = nc.dram_tensor("RT", [NROWS, NF], F16, kind="ExternalInput")
    CHEB = nc.dram_tensor("CHEB", [NROWS, CHEB_COLS], F32, kind="ExternalInput")
    MCOL = nc.dram_tensor("MCOL", [NROWS, 1], F32, kind="ExternalInput")
    Wd = {nm: nc.dram_tensor(nm, list(shp), F16 if nm in W16 else F32,
                             kind="ExternalInput")
          for nm, shp in WEIGHT_SHAPES.items()}
    ACC_D = nc.dram_tensor("ACC", [NROWS, 1], F32, kind="ExternalOutput")

    v = nc.vector
    a = nc.scalar
    g = nc.gpsimd
    te = nc.tensor
    dma = nc.sync

    with tile.TileContext(nc) as tc, ExitStack() as ctx:
        cpool = ctx.enter_context(tc.tile_pool(name="consts", bufs=1))
        pw = ctx.enter_context(tc.tile_pool(name="pw", bufs=2))
        ft = ctx.enter_context(tc.tile_pool(name="ft", bufs=3))
        rhp = ctx.enter_context(tc.tile_pool(name="rhp", bufs=1))
        hsp = ctx.enter_context(tc.tile_pool(name="hsp", bufs=1))
        drp = ctx.enter_context(tc.tile_pool(name="drp", bufs=2, space="DRAM"))
        psA = ctx.enter_context(tc.tile_pool(name="psA", bufs=1, space="PSUM"))
        psS = ctx.enter_context(tc.tile_pool(name="psS", bufs=2, space="PSUM"))
        psC = ctx.enter_context(tc.tile_pool(name="psC", bufs=2, space="PSUM"))
        psTB = ctx.enter_context(tc.tile_pool(name="psTB", bufs=1, space="PSUM"))
        psH = ctx.enter_context(tc.tile_pool(name="psH", bufs=1, space="PSUM"))

        W = {}
        for nm in Wd:
            W[nm] = cpool.tile(list(WEIGHT_SHAPES[nm]),
                               F16 if nm in W16 else F32,
                               name=f"w_{nm}", tag=f"w_{nm}")
            dma.dma_start(W[nm][:], Wd[nm][:])
        CH = cpool.tile([NROWS, CHEB_COLS], F32, name="cheb", tag="cheb")
        dma.dma_start(CH[:], CHEB[:])
        MC = cpool.tile([NROWS, 1], F32, name="mcol", tag="mcol")
        dma.dma_start(MC[:], MCOL[:])

        def chb(i):
            return CH[:, i:i + 1]

        def body():
            acc_parts = []

            # ---------- pointwise geometry + cheb for one chunk ----------
            def pw_phase(c, npc, cs):
                def pwt(tag, dt=F32):
                    return pw.tile([NROWS, npc], dt, name=tag, tag=tag)

                env = {"c": c, "npc": npc}
                X16, Y16, Z16, R16 = (pw.tile([NROWS, npc], F16, name=t, tag=t)
                                      for t in ("X16", "Y16", "Z16", "R16"))
                dma.dma_start(X16[:], X[:, cs])
                dma.dma_start(Y16[:], Y[:, cs])
                dma.dma_start(Z16[:], Z[:, cs])
                dma.dma_start(R16[:], RT[:, cs])

                s_a, s_b, s_c = pwt("s_a"), pwt("s_b"), pwt("s_c")
                D1t = pw.tile([NROWS, npc], F16, name="D1t", tag="D1t")
                D2t = pw.tile([NROWS, npc], F16, name="D2t", tag="D2t")
                YZ2 = pwt("YZ2")
                g.tensor_sub(D1t[:], X16[:], R16[:])
                g.tensor_add(D2t[:], X16[:], R16[:])
                a.square(s_a[:], Y16[:])
                a.square(s_b[:], Z16[:])
                g.tensor_add(YZ2[:], s_a[:], s_b[:])
                R1t, R2t = pwt("R1t"), pwt("R2t")
                a.square(s_a[:], D1t[:])
                g.tensor_add(s_a[:], s_a[:], YZ2[:])
                a.sqrt(R1t[:], s_a[:])
                a.square(s_b[:], D2t[:])
                g.tensor_add(s_b[:], s_b[:], YZ2[:])
                a.sqrt(R2t[:], s_b[:])
                Q1t, Q2t = pwt("Q1t"), pwt("Q2t")
                v.reciprocal_approx_fast(out=Q1t[:], in_=R1t[:])
                v.reciprocal_approx_fast(out=Q2t[:], in_=R2t[:])
                F1t, F2t = pwt("F1t"), pwt("F2t")
                a.activation(F1t[:], R1t[:], AT.Exp, scale=-1.0)
                a.activation(F2t[:], R2t[:], AT.Exp, scale=-1.0)
                # FALL: [F1h | F2h | F2Ch] f16, feeds the band gather
                FALL = pw.tile([NROWS, 3 * npc], F16, name="FALL", tag="FALL")
                a.copy(FALL[:, 0:npc], F1t[:])
                v.tensor_copy(FALL[:, npc:2 * npc], F2t[:])
                # c12 = (D1*D2 + YZ2) * Q1 * Q2 ; F2C = F2 * c12
                g.tensor_mul(s_a[:], D1t[:], D2t[:])
                g.tensor_add(s_a[:], s_a[:], YZ2[:])
                v.tensor_mul(s_b[:], Q1t[:], Q2t[:])
                g.tensor_mul(s_c[:], s_a[:], s_b[:])
                g.tensor_mul(FALL[:, 2 * npc:3 * npc], F2t[:], s_c[:])
                env.update(F1t=F1t, F2t=F2t, FALL=FALL, R16=R16,
                           Q1t=Q1t, Q2t=Q2t)
                return env

            # assembly-only pointwise work, emitted after the gather so the
            # bands start sooner and this fills engine idle during them
            def pw_aux(env):
                npc = env["npc"]
                F1t, F2t, FALL = env["F1t"], env["F2t"], env["FALL"]
                Q1t, Q2t, R16 = env["Q1t"], env["Q2t"], env["R16"]

                def pwt(tag, dt=F32):
                    return pw.tile([NROWS, npc], dt, name=tag, tag=tag)

                s_a, s_b = pwt("as_b"), pwt("as_c")
                W2SSt = pwt("W2SSt")
                a.square(s_a[:], F2t[:])
                a.square(s_b[:], FALL[:, 2 * npc:3 * npc])
                g.tensor_sub(W2SSt[:], s_a[:], s_b[:])
                S1Lt, S2Lt = pwt("S1Lt"), pwt("S2Lt")
                v.tensor_scalar(s_a[:], Q1t[:], -2.0, 1.0, OP.mult, OP.add)
                g.tensor_mul(S1Lt[:], s_a[:], F1t[:])
                v.tensor_scalar(s_b[:], Q2t[:], -2.0, 1.0, OP.mult, OP.add)
                g.tensor_mul(S2Lt[:], s_b[:], F2t[:])
                POTEt = pwt("POTEt")
                g.tensor_add(POTEt[:], Q1t[:], Q2t[:])

                RN = pwt("RN")
                v.tensor_scalar(RN[:], R16[:], chb(0), chb(1), OP.mult, OP.add)
                EEt, DECt = pwt("as_b"), pwt("DECt")

                def horner(eng, out, base, deg):
                    eng.tensor_scalar_mul(out[:], RN[:], chb(base + deg))
                    for k in range(deg - 1, 0, -1):
                        eng.scalar_tensor_tensor(out[:], out[:], chb(base + k),
                                                 RN[:], OP.add, OP.mult)
                    eng.tensor_scalar_add(out[:], out[:], chb(base))

                horner(v, EEt, 3, DEG_E)
                horner(v, DECt, 3 + DEG_E + 1, DEG_D)
                v.tensor_add(POTEt[:], POTEt[:], EEt[:])
                env.update(W2SSt=W2SSt, S1Lt=S1Lt, S2Lt=S2Lt, POTEt=POTEt,
                           DECt=DECt)
                return env

            # ---------- band-layout gather via DRAM bounce ----------
            # RHall[4q+r, b*npc+c] = FALL[4b+r, q*npc+c].  SBUF-side APs are
            # plain (dep tracking mishandles strided SBUF reads); the index
            # permutation runs in DRAM->DRAM legs (one per q, 3-dim APs).
            def gather_a(env):
                npc = env["npc"]
                DFA = drp.tile([NROWS, 3 * npc], F16, name="DFA", tag="DFA")
                dma.dma_start(DFA[:], env["FALL"][:])
                DFB = drp.tile([12, NBANDS * npc], F16, name="DFB", tag="DFB")
                dfa_q = DFA[:].rearrange("(b r) (q c) -> q r b c",
                                         b=NBANDS, q=3)
                dfb_q = DFB[:].rearrange("(q r) (b c) -> q r b c",
                                         q=3, b=NBANDS)
                for q in range(3):
                    dma.dma_start(dfb_q[q], dfa_q[q])
                env["DFB"] = DFB

            def gather_b(env):
                npc = env["npc"]
                RHall = rhp.tile([12, NBANDS * npc], F16, name="RHall",
                                 tag="RHall")
                dma.dma_start(RHall[:], env["DFB"][:])
                env["RHall"] = RHall

            # ---------- feature bands (3-stage software pipeline) ----------
            def band_s1(env, b):
                npc = env["npc"]
                rh = env["RHall"][:, b * npc:(b + 1) * npc]

                def ftt(tag, w=1):
                    return ft.tile([128, w * npc], F16, name=tag, tag=tag)

                pA = psA.tile([128, 512], F32, name="pA", tag="pA")
                te.matmul(pA[:, 0:npc], W["WA"][:], rh, start=True, stop=True)
                pS = psS.tile([128, 512], F32, name="pS", tag="pS")
                te.matmul(pS[:, 0:npc], W["WS"][:], rh, start=True, stop=True)
                TT1 = ftt("TT1", 2)   # [T1 | TAU1]
                TAU1 = TT1[:, npc:2 * npc]
                a.activation(TAU1, pA[:, 0:npc], AT.Tanh,
                             bias=W["BT1"][:, 0:1], scale=0.5)
                TSQ1 = ftt("TSQ1")
                v.tensor_mul(TSQ1[:], TAU1, TAU1)
                SP1N = ftt("SP1N")    # tsq1 - 1 = -4 sig'(z1)
                v.tensor_scalar_sub(SP1N[:], TSQ1[:], 1.0)
                # T1 = (tsq1 - 1) * pS
                v.scalar_tensor_tensor(TT1[:, 0:npc], TSQ1[:], 1.0,
                                       pS[:, 0:npc], OP.subtract, OP.mult)
                SQS = ftt("SQS")      # (s1/4)^2
                a.square(SQS[:], pS[:, 0:npc])
                return dict(b=b, TT1=TT1, TAU1=TAU1, TSQ1=TSQ1, SP1N=SP1N,
                            SQS=SQS)

            def band_s2(env, st):
                npc = env["npc"]

                def ftt(tag, w=1):
                    return ft.tile([128, w * npc], F16, name=tag, tag=tag)

                pTB = psTB.tile([128, 1024], F32, name="pTB", tag="pTB")
                te.matmul(pTB[:, 0:npc], W["W2bd"][:], st["TT1"][:, 0:npc],
                          start=True, stop=True)
                te.matmul(pTB[:, 512:512 + npc], W["W2bd"][:],
                          st["TT1"][:, npc:2 * npc], start=True, stop=True)
                TAU2 = ftt("TAU2")
                a.activation(TAU2[:], pTB[:, 512:512 + npc], AT.Tanh,
                             bias=W["BT2"][:, 0:1], scale=0.25)
                # pD2 reuses psTB bank 1 once TAU2 has consumed pB; for the
                # 512-wide chunk one Act square covers [pT | pD2] contiguously
                te.matmul(pTB[:, 512:512 + npc], W["WD2bdN"][:],
                          st["SP1N"][:], start=True, stop=True)
                SQTD = ft.tile([128, 512 + npc], F16, name="SQTD",
                               tag="SQTD")
                if npc == 512:
                    a.square(SQTD[:], pTB[:, 0:512 + npc])
                else:
                    a.square(SQTD[:, 0:npc], pTB[:, 0:npc])
                    a.square(SQTD[:, 512:512 + npc], pTB[:, 512:512 + npc])
                SQT = SQTD[:, 0:npc]
                SQD = SQTD[:, 512:512 + npc]
                TSQ2 = ftt("TSQ2")
                v.tensor_mul(TSQ2[:], TAU2[:], TAU2[:])
                UT = ftt("UT")        # (tsq2-1)*tau2 = 4 sig''(z2)
                v.scalar_tensor_tensor(UT[:], TSQ2[:], 1.0, TAU2[:],
                                       OP.subtract, OP.mult)
                pC = psC.tile([128, 512], F32, name="pC", tag="pC")
                te.matmul(pC[:, 0:npc], W["WCpos"][:], TSQ2[:],
                          start=True, stop=True)
                st.update(SQT=SQT, TAU2=TAU2, SQD=SQD, UT=UT, pC=pC)
                return st

            def band_s3(env, st):
                npc = env["npc"]
                b = st["b"]
                gi, j = b // 4, b % 4

                def ftt(tag, w=1):
                    return ft.tile([128, w * npc], F16, name=tag, tag=tag)

                VT2 = ftt("VT2")      # (pC' + ub0n)*sp1neg = ubar*sp1
                v.scalar_tensor_tensor(VT2[:], st["pC"][:, 0:npc],
                                       W["UB0N"][:, 0:1], st["SP1N"][:],
                                       OP.add, OP.mult)
                VT = ftt("VT")
                g.tensor_mul(VT[:], VT2[:], st["TAU1"])
                R1 = ftt("R1")
                v.tensor_mul(R1[:], st["UT"][:], st["SQT"])
                R2 = ftt("R2")
                v.tensor_mul(R2[:], VT[:], st["SQS"][:])
                R22 = ftt("R22")
                g.tensor_mul(R22[:], st["UT"][:], st["SQD"])

                if j == 0:
                    env["pH"] = psH.tile([128, 512], F32, name="pH", tag="pH")
                pH = env["pH"][32 * j:32 * j + 32, 0:npc]
                tp = (0, 32 * j)
                te.matmul(pH, W["HG"][:], st["TAU2"][:], start=True,
                          stop=False, tile_position=tp)
                te.matmul(pH, W["HVT2"][:], VT2[:], start=False,
                          stop=False, tile_position=tp)
                te.matmul(pH, W["HVT"][:], VT[:], start=False, stop=False,
                          tile_position=tp)
                te.matmul(pH, W["HR1"][:], R1[:], start=False, stop=False,
                          tile_position=tp)
                te.matmul(pH, W["HR2"][:], R2[:], start=False, stop=False,
                          tile_position=tp)
                te.matmul(pH, W["HR22"][:], R22[:], start=False, stop=True,
                          tile_position=tp)
                if j == 3:
                    a.copy(env["HST"][:, gi * npc:(gi + 1) * npc],
                           env["pH"][:, 0:npc])

            def bands(env):
                npc = env["npc"]
                env["HST"] = hsp.tile([NROWS, NGROUPS * npc], F16, name="HST",
                                      tag="HST")
                win = []
                for b in range(NBANDS):
                    win.append(band_s1(env, b))
                    if len(win) >= 3:
                        band_s3(env, win.pop(0))
                    if len(win) >= 2:
                        band_s2(env, win[-2])
                band_s2(env, win[-1])
                band_s3(env, win.pop(0))
                band_s3(env, win.pop(0))

            # ---------- head-output scatter via DRAM bounce ----------
            # GALL[16g+jp, h*npc+c] = HST[8*jp+h, g*npc+c]; permutation in
            # 5 per-head DRAM->DRAM legs, SBUF sides plain.
            def scatter(env):
                npc = env["npc"]
                DSA = drp.tile([NROWS, NGROUPS * npc], F16, name="DSA",
                               tag="DSA")
                dma.dma_start(DSA[:], env["HST"][:])
                DSB = drp.tile([NROWS, 5 * npc], F16, name="DSB", tag="DSB")
                dsa_h = DSA[:].rearrange("(jp e) (g c) -> e jp g c",
                                         e=8, g=NGROUPS)
                dsb_h = DSB[:].rearrange("(g jp) (h c) -> h jp g c",
                                         g=NGROUPS, h=5)
                for h in range(5):
                    dma.dma_start(dsb_h[h], dsa_h[h])
                GALL = pw.tile([NROWS, 5 * npc], F16, name="GALL", tag="GALL")
                dma.dma_start(GALL[:], DSB[:])
                env["GALL"] = GALL

            # ---------- assembly ----------
            def assembly(env):
                c, npc = env["c"], env["npc"]
                GALL = env["GALL"]
                Gh = GALL[:, 0:npc]
                G1h = GALL[:, npc:2 * npc]
                G2h = GALL[:, 2 * npc:3 * npc]
                HWh = GALL[:, 3 * npc:4 * npc]
                G22h = GALL[:, 4 * npc:5 * npc]

                def pwt(tag, dt=F32):
                    return pw.tile([NROWS, npc], dt, name=tag, tag=tag)

                s_a, s_b, s_c = pwt("as_a"), pwt("as_b"), pwt("as_c")
                PSIt, LAPt = pwt("PSIt"), pwt("LAPt")
                v.tensor_scalar_add(s_a[:], Gh, chb(2))
                g.tensor_mul(PSIt[:], s_a[:], env["DECt"][:])
                v.tensor_add(PSIt[:], PSIt[:], env["F1t"][:])
                v.tensor_add(PSIt[:], PSIt[:], env["F2t"][:])
                v.tensor_mul(s_a[:], env["W2SSt"][:], G22h)
                g.tensor_add(s_a[:], s_a[:], HWh)
                v.tensor_mul(s_b[:], G1h, env["S1Lt"][:])
                g.tensor_add(s_a[:], s_a[:], s_b[:])
                v.tensor_mul(s_c[:], G2h, env["S2Lt"][:])
                g.tensor_add(s_a[:], s_a[:], s_c[:])
                v.tensor_mul(LAPt[:], env["DECt"][:], s_a[:])
                g.tensor_add(LAPt[:], LAPt[:], env["S1Lt"][:])
                v.tensor_add(LAPt[:], LAPt[:], env["S2Lt"][:])
                REST = pwt("REST")
                v.tensor_mul(s_a[:], env["POTEt"][:], PSIt[:])
                v.scalar_tensor_tensor(REST[:], LAPt[:], -0.5, s_a[:],
                                       OP.mult, OP.subtract)
                if c == len(CHUNKS) - 1:
                    v.tensor_mul(REST[:, npc - 1:npc],
                                 REST[:, npc - 1:npc], MC[:, 0:1])
                acc_c = cpool.tile([NROWS, 1], F32, name=f"acc{c}",
                                   tag=f"acc{c}")
                a.activation(s_a[:], REST[:], AT.Square, accum_out=acc_c[:])
                acc_parts.append(acc_c)

            env0 = pw_phase(0, CHUNKS[0], slice(0, CHUNKS[0]))
            gather_a(env0)
            gather_b(env0)
            env1 = pw_phase(1, CHUNKS[1], slice(CHUNKS[0], NF))
            gather_a(env1)
            pw_aux(env0)
            bands(env0)
            scatter(env0)
            gather_b(env1)
            pw_aux(env1)
            assembly(env0)
            bands(env1)
            scatter(env1)
            assembly(env1)

            tot = cpool.tile([NROWS, 1], F32, name="acctot", tag="acctot")
            v.tensor_add(tot[:], acc_parts[0][:], acc_parts[1][:])
            dma.dma_start(ACC_D[:], tot[:])

        if bench_repeat > 1:
            with tc.For_i(0, bench_repeat, 1):
                body()
        else:
            body()

    nc.compile()
    return nc


def make_in_maps(inputs):
    params = {k: v for k, v in inputs.items() if k not in
              ("x", "y", "z", "R", "bIndex1", "bIndex2")}
    consts = build_consts(params)
    cheb = build_cheb(params, np.asarray(inputs["R"], np.float32))
    mcol = (np.arange(NROWS) < NVALID_LASTCOL).astype(np.float32)[:, None]

    in_maps = []
    for core in range(N_CORES):
        sl = slice(core * PER_CORE, (core + 1) * PER_CORE)

        def shard(arr, fill):
            s = np.asarray(arr, np.float32)[sl, 0]
            buf = np.full((NF, NROWS), fill, np.float32)
            buf.reshape(-1)[:PER_CORE] = s
            return np.ascontiguousarray(buf.T.astype(np.float16))

        m = dict(consts)
        m["X"] = shard(inputs["x"], 0.5)
        m["Y"] = shard(inputs["y"], 0.5)
        m["Z"] = shard(inputs["z"], 0.5)
        m["RT"] = shard(inputs["R"], 1.0)
        m["CHEB"] = cheb
        m["MCOL"] = mcol
        in_maps.append(m)
    return in_maps


def host_boundary(inputs):
    """Lbc = mean(psi[b1]^2) + mean(psi[b2]^2), float64 host computation."""
    p = {k: np.asarray(v, np.float64) for k, v in inputs.items()
         if k.startswith(("W_", "b_"))}
    idx = np.concatenate([np.asarray(inputs["bIndex1"]).astype(np.int64),
                          np.asarray(inputs["bIndex2"]).astype(np.int64)])
    x = np.asarray(inputs["x"], np.float64)[idx, 0]
    y = np.asarray(inputs["y"], np.float64)[idx, 0]
    z = np.asarray(inputs["z"], np.float64)[idx, 0]
    R = np.asarray(inputs["R"], np.float64)[idx, 0]
    r1 = np.sqrt((x - R) ** 2 + y ** 2 + z ** 2)
    r2 = np.sqrt((x + R) ** 2 + y ** 2 + z ** 2)
    f1, f2 = np.exp(-r1), np.exp(-r2)
    W1, b1 = p["W_H1"], p["b_H1"]
    W2, b2 = p["W_H2"], p["b_H2"]
    B = 0.0
    for (aa, bb) in ((f1, f2), (f2, f1)):
        h = _sigmoid(np.outer(aa, W1[:, 0]) + np.outer(bb, W1[:, 1]) + b1)
        B = B + _sigmoid(h @ W2.T + b2)
    fd = _sigmoid(np.outer(R, p["W_DL"][:, 0]) + p["b_DL"])
    dec = fd @ p["W_D"][0] + p["b_D"][0]
    psi = ((B @ p["W_out"][0]) + p["b_out"][0]) * dec + f1 + f2
    n = idx.shape[0] // 2
    return float((psi[:n] ** 2).mean() + (psi[n:] ** 2).mean())


_NC_CACHE = {}


def kernel(**inputs):
    if "nc" not in _NC_CACHE:
        _NC_CACHE["nc"] = build_bass()
    nc = _NC_CACHE["nc"]

    in_maps = make_in_maps(inputs)
    results = run_bass_kernel_spmd(nc, in_maps, core_ids=list(range(N_CORES)))
    outs = results.results

    res2 = float(sum(np.asarray(outs[c]["ACC"], np.float64).sum()
                     for c in range(N_CORES)))
    loss = res2 / N_TOTAL + host_boundary(inputs)
    return np.float32(loss)
